# revision 2
# baseline (speedup 1.0000x reference)
"""Trainium2 Bass kernel for ChannelCrissCrossAttention (v2, all on-chip).

Shapes (hardcoded): B=8, IN=128, C=16, V=T=64.
Sharding: pure data parallel, one batch element per NeuronCore (8 cores).

Math (see baseline derivation):
  q,k,v = conv3x3(x; wq/wk/wv) -> [C, V, T] per batch element (a=v, b=t).
  Row grids r=(c,a): G_r[x,j] = exp(q[c,a,x]*k[c,a,j]) serve tt (natural)
  and vv (spatial transpose). cc grids live on 16-elem strips of the flat
  channel-major layout at spatially-transposed positions.
  Z = S_tt + S_vv + S_cc; outputs W_*/Z; stacked reverse conv; gamma*out+x.

v2 vs baseline: no HBM scratch at all. qkv stays in SBUF; cc operands are
gathered by SBUF->SBUF rearrange DMAs; cc results return to [c, v, t] via
PE transposes (not f32 DMA transposes, which lower to 4-byte-descriptor
gathers and dominated the baseline's 21ms).
"""

import sys

sys.path.insert(0, "/opt/trn_rl_repo")

import numpy as np

import concourse.bass as bass
import concourse.tile as tile
from concourse import bacc, mybir
from concourse.bass_utils import run_bass_kernel_spmd

F32 = mybir.dt.float32
AF = mybir.ActivationFunctionType
ALU = mybir.AluOpType
AX = mybir.AxisListType

IN, C, V, T = 128, 16, 64, 64
CH3 = 3 * C  # 48
NPOS = V * T  # 4096
PW = 66  # padded spatial width


def _build_program(niter=1):
    nc = bacc.Bacc("TRN2", target_bir_lowering=False, debug=False)

    x_d = nc.dram_tensor("x", [IN, V, T], F32, kind="ExternalInput")
    wqkv_d = nc.dram_tensor("wqkv", [IN, 9 * CH3], F32, kind="ExternalInput")
    bqkv_d = nc.dram_tensor("bqkv", [CH3, 1], F32, kind="ExternalInput")
    wr_d = nc.dram_tensor("wr", [CH3, 9 * IN], F32, kind="ExternalInput")
    gb_d = nc.dram_tensor("gb", [IN, 1], F32, kind="ExternalInput")
    id_d = nc.dram_tensor("ident", [128, 128], F32, kind="ExternalInput")
    out_d = nc.dram_tensor("out", [IN, V, T], F32, kind="ExternalOutput")
    qkv_h = nc.dram_tensor("qkv_h", [CH3, V, T], F32)
    st_h = nc.dram_tensor("st_h", [2 * C, V, T], F32)
    ocat_h = nc.dram_tensor("ocat_h", [CH3, V, T], F32)

    with tile.TileContext(nc) as tc:
        for _ in range(niter):
            _body(nc, tc, x_d, wqkv_d, bqkv_d, wr_d, gb_d, id_d, out_d,
                  qkv_h, st_h, ocat_h)

    nc.compile()
    return nc


def _body(nc, tc, x_d, wqkv_d, bqkv_d, wr_d, gb_d, id_d, out_d,
          qkv_h, st_h, ocat_h):
    from contextlib import ExitStack
    ctx = ExitStack()
    persist = ctx.enter_context(tc.tile_pool(name="persist", bufs=1))
    pio = ctx.enter_context(tc.tile_pool(name="pio", bufs=2))
    pP = ctx.enter_context(tc.tile_pool(name="pP", bufs=2))
    pG = ctx.enter_context(tc.tile_pool(name="pG", bufs=2))
    pcomb = ctx.enter_context(tc.tile_pool(name="pcomb", bufs=2))
    psum1 = ctx.enter_context(
        tc.tile_pool(name="psum1", bufs=2, space=bass.MemorySpace.PSUM))
    psum2 = ctx.enter_context(
        tc.tile_pool(name="psum2", bufs=2, space=bass.MemorySpace.PSUM))
    psumT = ctx.enter_context(
        tc.tile_pool(name="psumT", bufs=1, space=bass.MemorySpace.PSUM))

    # ---- Phase 0: weights + padded input ----
    wqkv = persist.tile([IN, 9 * CH3], F32)
    nc.sync.dma_start(wqkv[:], wqkv_d.ap())
    bqkv = persist.tile([CH3, 1], F32)
    nc.sync.dma_start(bqkv[:], bqkv_d.ap())
    wr = persist.tile([CH3, 9 * IN], F32)
    nc.sync.dma_start(wr[:], wr_d.ap())
    gb = persist.tile([IN, 1], F32)
    nc.sync.dma_start(gb[:], gb_d.ap())
    ident = persist.tile([128, 128], F32)
    nc.sync.dma_start(ident[:], id_d.ap())

    xpad = persist.tile([IN, PW * PW], F32)
    nc.gpsimd.memset(xpad[:], 0.0)
    xpad_v = xpad[:].rearrange("p (v t) -> p v t", v=PW)
    for m in range(8):
        nc.sync.dma_start(
            xpad_v[:, 1 + m * 8:1 + (m + 1) * 8, 1:1 + T],
            x_d.ap()[:, m * 8:(m + 1) * 8, :])

    # ---- Phase 1: qkv conv -> qkv_h ----
    for m in range(8):
        ps = psum1.tile([CH3, 512], F32, tag="ps1")
        for tap in range(9):
            dy, dx = tap // 3, tap % 3
            rhs = xpad_v[:, m * 8 + dy: m * 8 + dy + 8, dx: dx + T]
            nc.tensor.matmul(
                ps[:], wqkv[:, tap * CH3:(tap + 1) * CH3], rhs,
                start=(tap == 0), stop=(tap == 8))
        qkv_t = pio.tile([CH3, 512], F32, tag="qkv_t")
        nc.scalar.activation(qkv_t[:], ps[:], AF.Identity, bias=bqkv[:])
        nc.sync.dma_start(
            qkv_h.ap()[:, m * 8:(m + 1) * 8, :],
            qkv_t[:].rearrange("p (v t) -> p v t", v=8))

    # ---- Phase 2: batched spreads from DRAM ----
    # Branch A operands: [(c2 v), b, t] <- qkv_h[2b+c2, v, t]
    QA = persist.tile([128, 8, T], F32)
    KA = persist.tile([128, 8, T], F32)
    VA = persist.tile([128, 8, T], F32)
    for i, dst in enumerate((QA, KA, VA)):
        src = qkv_h.ap()[i * C:(i + 1) * C]  # [16, 64, 64]
        for c2 in range(2):
            nc.sync.dma_start(
                dst[c2 * 64:(c2 + 1) * 64, :, :],
                src.rearrange("(b c2) v t -> c2 v b t", c2=2)[c2])

    # cc operands, slot pi = a*64+b -> partition P = (a%2)*64+b, chunk ah=a//2.
    # q/k strips sit at spatially transposed positions p=b*64+a: flat offset
    # 1024*(b%4)+4096*(b//4) + 32*ah + 16*(a%2) + j = 1024*b' + 32*ah + 16*al + j
    q_cc = persist.tile([128, 32, C], F32)
    k_cc = persist.tile([128, 32, C], F32)
    v_cc = persist.tile([128, 32, C], F32)
    for i, dst in ((0, q_cc), (1, k_cc)):
        flat = qkv_h.ap()[i * C:(i + 1) * C].flatten()  # [65536]
        srcv = flat.rearrange("(p64 ah al j) -> al p64 ah j",
                              p64=64, al=2, ah=32, j=C)
        # strip(b,a) base = 1024*(bh*4+bl) + 16*al -> p64 stride 1024: but
        # p64 enumerates (bh,bl) merged = b exactly; dims: [al][b:64][ah][j]
        for al in range(2):
            nc.sync.dma_start(dst[al * 64:(al + 1) * 64, :, :], srcv[al])
    # v strips natural: offset 16*pi = 16*(a*64+b) = 2048*ah + 1024*al + 16*b
    vflat = qkv_h.ap()[2 * C:3 * C].flatten()
    vsv = vflat.rearrange("(ah al b j) -> al b ah j", ah=32, al=2, b=64, j=C)
    for al in range(2):
        nc.sync.dma_start(v_cc[al * 64:(al + 1) * 64, :, :], vsv[al])

    # ---- Phase 3: cc compute ----
    S_cc = persist.tile([128, 32, C], F32)
    W_cc = persist.tile([128, 32, C], F32)
    NPH = 8
    for chk in range(32 // NPH):
        sl = slice(chk * NPH, (chk + 1) * NPH)
        qs = q_cc[:, sl, :]
        ks = k_cc[:, sl, :]
        vs = v_cc[:, sl, :]
        Pc = pP.tile([128, NPH, C, C], F32, tag="P")
        nc.vector.tensor_mul(
            Pc[:],
            qs.unsqueeze(3).broadcast_to([128, NPH, C, C]),
            ks.unsqueeze(2).broadcast_to([128, NPH, C, C]))
        Gc = pG.tile([128, NPH, C, C], F32, tag="G")
        nc.scalar.activation(Gc[:], Pc[:], AF.Exp)
        nc.vector.tensor_reduce(S_cc[:, sl, :], Gc[:], axis=AX.X, op=ALU.add)
        nc.vector.tensor_mul(
            Pc[:], Gc[:], vs.unsqueeze(2).broadcast_to([128, NPH, C, C]))
        nc.vector.tensor_reduce(W_cc[:, sl, :], Pc[:], axis=AX.X, op=ALU.add)

    # ---- Phase 4: PE back-transpose [128,16] chunks -> S_T/W_T [16, 4096] ----
    S_T = persist.tile([C, NPOS], F32)
    W_T = persist.tile([C, NPOS], F32)
    for dst_sb, src_cc, nm in ((S_T, S_cc, "S"), (W_T, W_cc, "W")):
        for g in range(8):  # 4 chunks per PSUM tile
            pt = psum2.tile([C, 512], F32, tag="ps2")
            for i in range(4):
                k = g * 4 + i
                nc.tensor.matmul(
                    pt[:, i * 128:(i + 1) * 128], src_cc[:, k, :], ident[:],
                    is_transpose=True)
            nc.scalar.copy(dst_sb[:, g * 512:(g + 1) * 512], pt[:])

    # combine-side spreads: S_T/W_T -> DRAM (contiguous) -> [(c2 v), b, t]
    nc.sync.dma_start(st_h.ap()[0:C], S_T[:].rearrange("c (v t) -> c v t", v=V))
    nc.sync.dma_start(st_h.ap()[C:2 * C],
                      W_T[:].rearrange("c (v t) -> c v t", v=V))
    ScA = persist.tile([128, 8, T], F32)
    WcA = persist.tile([128, 8, T], F32)
    for i, dst in ((0, ScA), (1, WcA)):
        src = st_h.ap()[i * C:(i + 1) * C]  # [16, 64, 64]
        for c2 in range(2):
            nc.sync.dma_start(
                dst[c2 * 64:(c2 + 1) * 64, :, :],
                src.rearrange("(b c2) v t -> c2 v b t", c2=2)[c2])

    # ---- Phase 5: branch A + combine ----
    S_all = persist.tile([128, 8 * T], F32)
    W_all = persist.tile([128, 8 * T], F32)

    for b in range(8):
        Qb = QA[:, b, :]
        Kb = KA[:, b, :]
        Vb = VA[:, b, :]

        P = pP.tile([128, T, T], F32, tag="P")
        nc.vector.tensor_mul(
            P[:],
            Qb.unsqueeze(2).broadcast_to([128, T, T]),
            Kb.unsqueeze(1).broadcast_to([128, T, T]))
        G = pG.tile([128, T, T], F32, tag="G")
        nc.scalar.activation(G[:], P[:], AF.Exp)
        Sb = S_all[:, b * T:(b + 1) * T]
        Wb = W_all[:, b * T:(b + 1) * T]
        nc.vector.tensor_reduce(Sb, G[:], axis=AX.X, op=ALU.add)
        nc.vector.tensor_mul(
            P[:], G[:], Vb.unsqueeze(1).broadcast_to([128, T, T]))
        nc.vector.tensor_reduce(Wb, P[:], axis=AX.X, op=ALU.add)

        # vv terms via PE transposes (each c2 half base-0 in its PSUM tile)
        S2h = [psumT.tile([T, T], F32, tag=f"S2{c2}", name=f"S2h{c2}_{b}")
               for c2 in range(2)]
        W2h = [psumT.tile([T, T], F32, tag=f"W2{c2}", name=f"W2h{c2}_{b}")
               for c2 in range(2)]
        for c2 in range(2):
            rows = slice(c2 * 64, (c2 + 1) * 64)
            nc.tensor.matmul(S2h[c2][:], Sb[rows], ident[rows, rows],
                             is_transpose=True)
            nc.tensor.matmul(W2h[c2][:], Wb[rows], ident[rows, rows],
                             is_transpose=True)

        Z = pcomb.tile([128, T], F32, tag="Z")
        R = pcomb.tile([128, T], F32, tag="R")
        for c2 in range(2):
            rows = slice(c2 * 64, (c2 + 1) * 64)
            nc.vector.tensor_add(Z[rows, :], Sb[rows], S2h[c2][:])
        nc.vector.tensor_add(Z[:], Z[:], ScA[:, b, :])
        nc.vector.reciprocal(R[:], Z[:])

        Ov = pcomb.tile([128, T], F32, tag="Ov")
        for c2 in range(2):
            rows = slice(c2 * 64, (c2 + 1) * 64)
            nc.vector.tensor_mul(Ov[rows, :], W2h[c2][:], R[rows, :])
        Oc = pcomb.tile([128, T], F32, tag="Oc")
        nc.vector.tensor_mul(Oc[:], WcA[:, b, :], R[:])
        Ot = pcomb.tile([128, T], F32, tag="Ot")
        nc.vector.tensor_mul(Ot[:], Wb, R[:])
        for t_, g in ((Oc, 0), (Ov, 1), (Ot, 2)):
            nc.sync.dma_start(
                ocat_h.ap()[g * C + 2 * b: g * C + 2 * b + 2], t_[:])

    # ---- Phase 6: reverse conv + residual ----
    opad = persist.tile([CH3, PW * PW], F32)
    nc.gpsimd.memset(opad[:], 0.0)
    opad_v = opad[:].rearrange("p (v t) -> p v t", v=PW)
    nc.sync.dma_start(opad_v[:, 1:1 + V, 1:1 + T], ocat_h.ap())

    for m in range(8):
        ps2 = psum2.tile([IN, 512], F32, tag="ps2")
        for tap in range(9):
            dy, dx = tap // 3, tap % 3
            rhs = opad_v[:, m * 8 + dy: m * 8 + dy + 8, dx: dx + T]
            nc.tensor.matmul(
                ps2[:], wr[:, tap * IN:(tap + 1) * IN], rhs,
                start=(tap == 0), stop=(tap == 8))
        o_sb = pio.tile([IN, 512], F32, tag="o_sb")
        xin = xpad_v[:, m * 8 + 1: m * 8 + 9, 1:1 + T]
        nc.vector.scalar_tensor_tensor(
            o_sb[:].rearrange("p (v t) -> p v t", v=8),
            ps2[:].rearrange("p (v t) -> p v t", v=8),
            gb[:], xin, op0=ALU.add, op1=ALU.add)
        nc.sync.dma_start(out_d.ap()[:, m * 8:(m + 1) * 8, :],
                          o_sb[:].rearrange("p (v t) -> p v t", v=8))

    ctx.close()


_NC_CACHE = {}


def _get_program(niter=1):
    if niter not in _NC_CACHE:
        _NC_CACHE[niter] = _build_program(niter)
    return _NC_CACHE[niter]


def _host_weights(wq, bq, wk, bk, wv, bv, wcr, bcr, wvr, bvr, wtr, btr, gamma):
    g = np.float32(np.asarray(gamma).reshape(-1)[0])
    wf = np.concatenate([wq, wk, wv], axis=0)  # [48, 128, 3, 3]
    wqkv = np.ascontiguousarray(
        wf.transpose(1, 2, 3, 0).reshape(IN, 9 * CH3)).astype(np.float32)
    bqkv = np.concatenate([bq, bk, bv]).reshape(CH3, 1).astype(np.float32)
    wrf = np.concatenate([wcr, wvr, wtr], axis=1) * g  # [128, 48, 3, 3]
    wr_ = np.ascontiguousarray(
        wrf.transpose(1, 2, 3, 0).reshape(CH3, 9 * IN)).astype(np.float32)
    gb = (g * (bcr + bvr + btr)).reshape(IN, 1).astype(np.float32)
    return wqkv, bqkv, wr_, gb


def kernel(x, wq, bq, wk, bk, wv, bv, wcr, bcr, wvr, bvr, wtr, btr, gamma,
           _trace=False, _niter=1):
    nc = _get_program(_niter)
    wqkv, bqkv, wr_, gb = _host_weights(
        wq, bq, wk, bk, wv, bv, wcr, bcr, wvr, bvr, wtr, btr, gamma)
    x = np.asarray(x, dtype=np.float32)
    ident = np.eye(128, dtype=np.float32)
    in_maps = [
        {"x": np.ascontiguousarray(x[i]), "wqkv": wqkv, "bqkv": bqkv,
         "wr": wr_, "gb": gb, "ident": ident}
        for i in range(8)
    ]
    res = run_bass_kernel_spmd(nc, in_maps, list(range(8)), trace=_trace)
    out = np.stack([res.results[i]["out"] for i in range(8)]).astype(np.float32)
    if _trace:
        kernel.last_exec_time_ns = res.exec_time_ns
        kernel.last_results = res
    return out


# revision 3
# speedup vs baseline: 48.8870x; 48.8870x over previous
"""Trainium2 Bass kernel for ChannelCrissCrossAttention (v2, all on-chip).

Shapes (hardcoded): B=8, IN=128, C=16, V=T=64.
Sharding: pure data parallel, one batch element per NeuronCore (8 cores).

Math (see baseline derivation):
  q,k,v = conv3x3(x; wq/wk/wv) -> [C, V, T] per batch element (a=v, b=t).
  Row grids r=(c,a): G_r[x,j] = exp(q[c,a,x]*k[c,a,j]) serve tt (natural)
  and vv (spatial transpose). cc grids live on 16-elem strips of the flat
  channel-major layout at spatially-transposed positions.
  Z = S_tt + S_vv + S_cc; outputs W_*/Z; stacked reverse conv; gamma*out+x.

v2 vs baseline: no HBM scratch at all. qkv stays in SBUF; cc operands are
gathered by SBUF->SBUF rearrange DMAs; cc results return to [c, v, t] via
PE transposes (not f32 DMA transposes, which lower to 4-byte-descriptor
gathers and dominated the baseline's 21ms).
"""

import sys

sys.path.insert(0, "/opt/trn_rl_repo")

import numpy as np

import concourse.bass as bass
import concourse.tile as tile
from concourse import bacc, mybir
from concourse.bass_utils import run_bass_kernel_spmd

F32 = mybir.dt.float32
AF = mybir.ActivationFunctionType
ALU = mybir.AluOpType
AX = mybir.AxisListType

IN, C, V, T = 128, 16, 64, 64
CH3 = 3 * C  # 48
NPOS = V * T  # 4096
PW = 66  # padded spatial width


def _build_program(niter=1):
    nc = bacc.Bacc("TRN2", target_bir_lowering=False, debug=False)

    x_d = nc.dram_tensor("x", [IN, V, T], F32, kind="ExternalInput")
    wqkv_d = nc.dram_tensor("wqkv", [IN, 9 * CH3], F32, kind="ExternalInput")
    bqkv_d = nc.dram_tensor("bqkv", [CH3, 1], F32, kind="ExternalInput")
    wr_d = nc.dram_tensor("wr", [CH3, 9 * IN], F32, kind="ExternalInput")
    gb_d = nc.dram_tensor("gb", [IN, 1], F32, kind="ExternalInput")
    id_d = nc.dram_tensor("ident", [128, 128], F32, kind="ExternalInput")
    out_d = nc.dram_tensor("out", [IN, V, T], F32, kind="ExternalOutput")
    qkv_h = nc.dram_tensor("qkv_h", [CH3, V, T], F32)
    st_h = nc.dram_tensor("st_h", [2 * C, V, T], F32)
    ocat_h = nc.dram_tensor("ocat_h", [CH3, V, T], F32)

    with tile.TileContext(nc) as tc:
        if niter == 1:
            _body(nc, tc, x_d, wqkv_d, bqkv_d, wr_d, gb_d, id_d, out_d,
                  qkv_h, st_h, ocat_h)
        else:
            with tc.For_i(0, niter, 1):
                _body(nc, tc, x_d, wqkv_d, bqkv_d, wr_d, gb_d, id_d, out_d,
                      qkv_h, st_h, ocat_h)

    nc.compile()
    return nc


def _body(nc, tc, x_d, wqkv_d, bqkv_d, wr_d, gb_d, id_d, out_d,
          qkv_h, st_h, ocat_h):
    from contextlib import ExitStack
    ctx = ExitStack()
    persist = ctx.enter_context(tc.tile_pool(name="persist", bufs=1))
    pio = ctx.enter_context(tc.tile_pool(name="pio", bufs=2))
    pP = ctx.enter_context(tc.tile_pool(name="pP", bufs=2))
    pG = ctx.enter_context(tc.tile_pool(name="pG", bufs=2))
    pcomb = ctx.enter_context(tc.tile_pool(name="pcomb", bufs=2))
    psum1 = ctx.enter_context(
        tc.tile_pool(name="psum1", bufs=2, space=bass.MemorySpace.PSUM))
    psum2 = ctx.enter_context(
        tc.tile_pool(name="psum2", bufs=2, space=bass.MemorySpace.PSUM))
    psumT = ctx.enter_context(
        tc.tile_pool(name="psumT", bufs=1, space=bass.MemorySpace.PSUM))

    # ---- Phase 0: weights + padded input ----
    wqkv = persist.tile([IN, 9 * CH3], F32)
    nc.sync.dma_start(wqkv[:], wqkv_d.ap())
    bqkv = persist.tile([CH3, 1], F32)
    nc.sync.dma_start(bqkv[:], bqkv_d.ap())
    wr = persist.tile([CH3, 9 * IN], F32)
    nc.sync.dma_start(wr[:], wr_d.ap())
    gb = persist.tile([IN, 1], F32)
    nc.sync.dma_start(gb[:], gb_d.ap())
    ident = persist.tile([128, 128], F32)
    nc.sync.dma_start(ident[:], id_d.ap())

    xpad = persist.tile([IN, PW * PW], F32)
    nc.gpsimd.memset(xpad[:], 0.0)
    xpad_v = xpad[:].rearrange("p (v t) -> p v t", v=PW)
    for m in range(8):
        nc.sync.dma_start(
            xpad_v[:, 1 + m * 8:1 + (m + 1) * 8, 1:1 + T],
            x_d.ap()[:, m * 8:(m + 1) * 8, :])

    # ---- Phase 1: qkv conv -> qkv_h ----
    for m in range(8):
        ps = psum1.tile([CH3, 512], F32, tag="ps1")
        for tap in range(9):
            dy, dx = tap // 3, tap % 3
            rhs = xpad_v[:, m * 8 + dy: m * 8 + dy + 8, dx: dx + T]
            nc.tensor.matmul(
                ps[:], wqkv[:, tap * CH3:(tap + 1) * CH3], rhs,
                start=(tap == 0), stop=(tap == 8))
        qkv_t = pio.tile([CH3, 512], F32, tag="qkv_t")
        nc.scalar.activation(qkv_t[:], ps[:], AF.Identity, bias=bqkv[:])
        nc.sync.dma_start(
            qkv_h.ap()[:, m * 8:(m + 1) * 8, :],
            qkv_t[:].rearrange("p (v t) -> p v t", v=8))

    # ---- Phase 2: batched spreads from DRAM ----
    # Branch A operands: [(c2 v), b, t] <- qkv_h[2b+c2, v, t]
    QA = persist.tile([128, 8, T], F32)
    KA = persist.tile([128, 8, T], F32)
    VA = persist.tile([128, 8, T], F32)
    for i, dst in enumerate((QA, KA, VA)):
        src = qkv_h.ap()[i * C:(i + 1) * C]  # [16, 64, 64]
        for c2 in range(2):
            nc.sync.dma_start(
                dst[c2 * 64:(c2 + 1) * 64, :, :],
                src.rearrange("(b c2) v t -> c2 v b t", c2=2)[c2])

    # cc operands, slot pi = a*64+b -> partition P = (a%2)*64+b, chunk ah=a//2.
    # q/k strips sit at spatially transposed positions p=b*64+a: flat offset
    # 1024*(b%4)+4096*(b//4) + 32*ah + 16*(a%2) + j = 1024*b' + 32*ah + 16*al + j
    q_cc = persist.tile([128, 32, C], F32)
    k_cc = persist.tile([128, 32, C], F32)
    v_cc = persist.tile([128, 32, C], F32)
    for i, dst in ((0, q_cc), (1, k_cc)):
        flat = qkv_h.ap()[i * C:(i + 1) * C].flatten()  # [65536]
        srcv = flat.rearrange("(p64 ah al j) -> al p64 ah j",
                              p64=64, al=2, ah=32, j=C)
        # strip(b,a) base = 1024*(bh*4+bl) + 16*al -> p64 stride 1024: but
        # p64 enumerates (bh,bl) merged = b exactly; dims: [al][b:64][ah][j]
        for al in range(2):
            nc.sync.dma_start(dst[al * 64:(al + 1) * 64, :, :], srcv[al])
    # v strips natural: offset 16*pi = 16*(a*64+b) = 2048*ah + 1024*al + 16*b
    vflat = qkv_h.ap()[2 * C:3 * C].flatten()
    vsv = vflat.rearrange("(ah al b j) -> al b ah j", ah=32, al=2, b=64, j=C)
    for al in range(2):
        nc.sync.dma_start(v_cc[al * 64:(al + 1) * 64, :, :], vsv[al])

    # ---- Phase 3: cc compute ----
    S_cc = persist.tile([128, 32, C], F32)
    W_cc = persist.tile([128, 32, C], F32)
    NPH = 8
    for chk in range(32 // NPH):
        sl = slice(chk * NPH, (chk + 1) * NPH)
        qs = q_cc[:, sl, :]
        ks = k_cc[:, sl, :]
        vs = v_cc[:, sl, :]
        Pc = pP.tile([128, NPH, C, C], F32, tag="P")
        nc.vector.tensor_mul(
            Pc[:],
            qs.unsqueeze(3).broadcast_to([128, NPH, C, C]),
            ks.unsqueeze(2).broadcast_to([128, NPH, C, C]))
        Gc = pG.tile([128, NPH, C, C], F32, tag="G")
        nc.scalar.activation(Gc[:], Pc[:], AF.Exp)
        nc.vector.tensor_reduce(S_cc[:, sl, :], Gc[:], axis=AX.X, op=ALU.add)
        nc.vector.tensor_mul(
            Pc[:], Gc[:], vs.unsqueeze(2).broadcast_to([128, NPH, C, C]))
        nc.vector.tensor_reduce(W_cc[:, sl, :], Pc[:], axis=AX.X, op=ALU.add)

    # ---- Phase 4: PE back-transpose [128,16] chunks -> S_T/W_T [16, 4096] ----
    S_T = persist.tile([C, NPOS], F32)
    W_T = persist.tile([C, NPOS], F32)
    for dst_sb, src_cc, nm in ((S_T, S_cc, "S"), (W_T, W_cc, "W")):
        for g in range(8):  # 4 chunks per PSUM tile
            pt = psum2.tile([C, 512], F32, tag="ps2")
            for i in range(4):
                k = g * 4 + i
                nc.tensor.matmul(
                    pt[:, i * 128:(i + 1) * 128], src_cc[:, k, :], ident[:],
                    is_transpose=True)
            nc.scalar.copy(dst_sb[:, g * 512:(g + 1) * 512], pt[:])

    # combine-side spreads: S_T/W_T -> DRAM (contiguous) -> [(c2 v), b, t]
    nc.sync.dma_start(st_h.ap()[0:C], S_T[:].rearrange("c (v t) -> c v t", v=V))
    nc.sync.dma_start(st_h.ap()[C:2 * C],
                      W_T[:].rearrange("c (v t) -> c v t", v=V))
    ScA = persist.tile([128, 8, T], F32)
    WcA = persist.tile([128, 8, T], F32)
    for i, dst in ((0, ScA), (1, WcA)):
        src = st_h.ap()[i * C:(i + 1) * C]  # [16, 64, 64]
        for c2 in range(2):
            nc.sync.dma_start(
                dst[c2 * 64:(c2 + 1) * 64, :, :],
                src.rearrange("(b c2) v t -> c2 v b t", c2=2)[c2])

    # ---- Phase 5: branch A + combine ----
    S_all = persist.tile([128, 8 * T], F32)
    W_all = persist.tile([128, 8 * T], F32)

    for b in range(8):
        Qb = QA[:, b, :]
        Kb = KA[:, b, :]
        Vb = VA[:, b, :]

        P = pP.tile([128, T, T], F32, tag="P")
        nc.vector.tensor_mul(
            P[:],
            Qb.unsqueeze(2).broadcast_to([128, T, T]),
            Kb.unsqueeze(1).broadcast_to([128, T, T]))
        G = pG.tile([128, T, T], F32, tag="G")
        nc.scalar.activation(G[:], P[:], AF.Exp)
        Sb = S_all[:, b * T:(b + 1) * T]
        Wb = W_all[:, b * T:(b + 1) * T]
        nc.vector.tensor_reduce(Sb, G[:], axis=AX.X, op=ALU.add)
        nc.vector.tensor_mul(
            P[:], G[:], Vb.unsqueeze(1).broadcast_to([128, T, T]))
        nc.vector.tensor_reduce(Wb, P[:], axis=AX.X, op=ALU.add)

        # vv terms via PE transposes (each c2 half base-0 in its PSUM tile)
        S2h = [psumT.tile([T, T], F32, tag=f"S2{c2}", name=f"S2h{c2}_{b}")
               for c2 in range(2)]
        W2h = [psumT.tile([T, T], F32, tag=f"W2{c2}", name=f"W2h{c2}_{b}")
               for c2 in range(2)]
        for c2 in range(2):
            rows = slice(c2 * 64, (c2 + 1) * 64)
            nc.tensor.matmul(S2h[c2][:], Sb[rows], ident[rows, rows],
                             is_transpose=True)
            nc.tensor.matmul(W2h[c2][:], Wb[rows], ident[rows, rows],
                             is_transpose=True)

        Z = pcomb.tile([128, T], F32, tag="Z")
        R = pcomb.tile([128, T], F32, tag="R")
        for c2 in range(2):
            rows = slice(c2 * 64, (c2 + 1) * 64)
            nc.vector.tensor_add(Z[rows, :], Sb[rows], S2h[c2][:])
        nc.vector.tensor_add(Z[:], Z[:], ScA[:, b, :])
        nc.vector.reciprocal(R[:], Z[:])

        Ov = pcomb.tile([128, T], F32, tag="Ov")
        for c2 in range(2):
            rows = slice(c2 * 64, (c2 + 1) * 64)
            nc.vector.tensor_mul(Ov[rows, :], W2h[c2][:], R[rows, :])
        Oc = pcomb.tile([128, T], F32, tag="Oc")
        nc.vector.tensor_mul(Oc[:], WcA[:, b, :], R[:])
        Ot = pcomb.tile([128, T], F32, tag="Ot")
        nc.vector.tensor_mul(Ot[:], Wb, R[:])
        for t_, g in ((Oc, 0), (Ov, 1), (Ot, 2)):
            nc.sync.dma_start(
                ocat_h.ap()[g * C + 2 * b: g * C + 2 * b + 2], t_[:])

    # ---- Phase 6: reverse conv + residual ----
    opad = persist.tile([CH3, PW * PW], F32)
    nc.gpsimd.memset(opad[:], 0.0)
    opad_v = opad[:].rearrange("p (v t) -> p v t", v=PW)
    nc.sync.dma_start(opad_v[:, 1:1 + V, 1:1 + T], ocat_h.ap())

    for m in range(8):
        ps2 = psum2.tile([IN, 512], F32, tag="ps2")
        for tap in range(9):
            dy, dx = tap // 3, tap % 3
            rhs = opad_v[:, m * 8 + dy: m * 8 + dy + 8, dx: dx + T]
            nc.tensor.matmul(
                ps2[:], wr[:, tap * IN:(tap + 1) * IN], rhs,
                start=(tap == 0), stop=(tap == 8))
        o_sb = pio.tile([IN, 512], F32, tag="o_sb")
        xin = xpad_v[:, m * 8 + 1: m * 8 + 9, 1:1 + T]
        nc.vector.scalar_tensor_tensor(
            o_sb[:].rearrange("p (v t) -> p v t", v=8),
            ps2[:].rearrange("p (v t) -> p v t", v=8),
            gb[:], xin, op0=ALU.add, op1=ALU.add)
        nc.sync.dma_start(out_d.ap()[:, m * 8:(m + 1) * 8, :],
                          o_sb[:].rearrange("p (v t) -> p v t", v=8))

    ctx.close()


_NC_CACHE = {}


def _get_program(niter=1):
    if niter not in _NC_CACHE:
        _NC_CACHE[niter] = _build_program(niter)
    return _NC_CACHE[niter]


def _host_weights(wq, bq, wk, bk, wv, bv, wcr, bcr, wvr, bvr, wtr, btr, gamma):
    g = np.float32(np.asarray(gamma).reshape(-1)[0])
    wf = np.concatenate([wq, wk, wv], axis=0)  # [48, 128, 3, 3]
    wqkv = np.ascontiguousarray(
        wf.transpose(1, 2, 3, 0).reshape(IN, 9 * CH3)).astype(np.float32)
    bqkv = np.concatenate([bq, bk, bv]).reshape(CH3, 1).astype(np.float32)
    wrf = np.concatenate([wcr, wvr, wtr], axis=1) * g  # [128, 48, 3, 3]
    wr_ = np.ascontiguousarray(
        wrf.transpose(1, 2, 3, 0).reshape(CH3, 9 * IN)).astype(np.float32)
    gb = (g * (bcr + bvr + btr)).reshape(IN, 1).astype(np.float32)
    return wqkv, bqkv, wr_, gb


def kernel(x, wq, bq, wk, bk, wv, bv, wcr, bcr, wvr, bvr, wtr, btr, gamma,
           _trace=False, _niter=1):
    nc = _get_program(_niter)
    wqkv, bqkv, wr_, gb = _host_weights(
        wq, bq, wk, bk, wv, bv, wcr, bcr, wvr, bvr, wtr, btr, gamma)
    x = np.asarray(x, dtype=np.float32)
    ident = np.eye(128, dtype=np.float32)
    in_maps = [
        {"x": np.ascontiguousarray(x[i]), "wqkv": wqkv, "bqkv": bqkv,
         "wr": wr_, "gb": gb, "ident": ident}
        for i in range(8)
    ]
    res = run_bass_kernel_spmd(nc, in_maps, list(range(8)), trace=_trace)
    out = np.stack([res.results[i]["out"] for i in range(8)]).astype(np.float32)
    if _trace:
        kernel.last_exec_time_ns = res.exec_time_ns
        kernel.last_results = res
    return out
